# revision 17
# baseline (speedup 1.0000x reference)
"""Circulant 1x1 conv (nn_Circulant1x1Conv) as a Trainium2 Bass kernel.

Math: the reference does, per spatial position r (N = batch*h*w rows):
    y[r, s*C + n] = irfft(rfft(x[r, :]) * cf[s])[n]  (circular convolution)
which is exactly a matmul  Y(N, 2048) = X(N, 512) @ W(512, 2048)  with
    W[k, s*C + n] = c_s[(n - k) mod C],   c_s = irfft(cf[s], n=C).

Crucially the native memory layouts are already transposed the right way:
  x[b] viewed as (C=512, h*w=1024) is X^T for that batch, and the output
  (nstack*C=2048, h*w) per batch is Y^T. So per batch:
      Out_b (2048, hw) = W^T @ X_b  ==  matmul(out, lhsT=W, rhs=X_b)
  on the tensor engine with zero data transposes anywhere.

Sharding: data-parallel over batch, 4 batches per core x 8 cores. Each core
computes a (2048, 4096) = (512, 2048)^T @ (512, 4096) matmul.

Precision knob DT_KIND:
  - "f32r": fp32 data, PE in fp32r (replicated/TF32-like) mode: 1 cycle/row
            at free-dim >= 256 per the cost model -> bf16-speed w/ fp32 inputs.
  - "bf16": inputs cast to bf16 on host; ~5e-3 rel error.
  - "f32":  exact fp32 matmul, 4 cycles/row (slow; debugging fallback).
"""

import numpy as np

SIZE = 512          # channels C (circulant size)
NSTACK = 4
BATCH = 32
HW = 32 * 32
N_CORES = 8
BPC = BATCH // N_CORES          # batches per core = 4
COLS = BPC * HW                 # moving free dim per core = 4096
M_OUT = NSTACK * SIZE           # output channels = 2048
P = 128
KC = SIZE // P                  # contraction chunks = 4
MT = M_OUT // P                 # output row tiles = 16
NFREE = 512                     # matmul moving free dim (1 PSUM bank fp32)
NT = COLS // NFREE              # moving chunks = 8
GN = 4                          # psum tiles per group (half of PSUM banks)
NG = NT // GN                   # groups per m-tile = 2

DT_KIND = "bf16"
OUT_BF16 = True     # DMA outputs as bf16 (half the write traffic); host upcasts

_CACHE = {}


def _build_nc(dt_kind, out_bf16=OUT_BF16):
    import concourse.bacc as bacc
    import concourse.tile as tile
    from concourse import mybir

    io_dt = {"bf16": mybir.dt.bfloat16,
             "f32r": mybir.dt.float32r,
             "f32": mybir.dt.float32}[dt_kind]
    out_dt = mybir.dt.bfloat16 if out_bf16 else mybir.dt.float32

    nc = bacc.Bacc("TRN2", name="circulant1x1")
    x = nc.dram_tensor("x", [SIZE, COLS], io_dt, kind="ExternalInput")
    w = nc.dram_tensor("w", [SIZE, M_OUT], io_dt, kind="ExternalInput")
    out = nc.dram_tensor("out", [M_OUT, COLS], out_dt,
                         kind="ExternalOutput")

    with tile.TileContext(nc) as tc:
        with (
            tc.tile_pool(name="junk", bufs=1) as jp,
            tc.tile_pool(name="xin", bufs=1) as xp,
            tc.tile_pool(name="win", bufs=1) as wp,
            tc.tile_pool(name="outp", bufs=8) as op,
            tc.tile_pool(name="outpt", bufs=4) as opt,
            tc.tile_pool(name="ps", bufs=8, space="PSUM") as pp,
        ):
            HCOL = COLS // NG                   # columns per group = 2048
            x_sb = xp.tile([P, KC, COLS], io_dt)
            w_sb = wp.tile([P, KC, M_OUT], io_dt)

            # HAM warmup on a memset junk tile, gated on nothing: the PE
            # is busy from the moment the tile-context preamble ends
            # (~8.4us incl. the memset) and stays busy until the first
            # real matmul's data lands (~10.6us). The HAM busy-clock
            # (full speed after ~5us of continuous activity) then expires
            # during the ramp phase.
            junk = jp.tile([P, NFREE], io_dt)
            nc.vector.memset(junk[:], 0.0)
            for i in range(6):
                wps = pp.tile([P, NFREE], mybir.dt.float32, tag="ps",
                              name=f"warm_{i}")
                nc.tensor.matmul(wps, junk[:, 0:P], junk[:],
                                 start=True, stop=True)

            # Input DMAs: the sync ring is measurably faster to start than
            # the scalar ring (whose first trigger waits for an
            # ACT_TABLE_LOAD), so every ramp-critical piece rides sync, in
            # exact consumption order, smallest-first so the first real
            # matmul (m0 g0 j0 k0: w k0 cols 0:256 + x k0 cols 0:1024) is
            # gated on just 320KB. The scalar ring carries the rest of w
            # in parallel; bulk pieces are merged into single big DMAs
            # (each dma_start costs ~600ns of issuing-engine time).
            WRAMP = 2 * P                       # ramp rows m0/m1
            WR = 4 * P                          # m0..m3 weight columns
            HH = HCOL // 2
            nc.sync.dma_start(out=w_sb[:, 0, 0:WRAMP], in_=w[0:P, 0:WRAMP])
            nc.sync.dma_start(out=x_sb[:, 0, 0:HH], in_=x[0:P, 0:HH])
            nc.sync.dma_start(out=x_sb[:, 0, HH:HCOL], in_=x[0:P, HH:HCOL])
            for k in range(1, KC):
                nc.sync.dma_start(out=x_sb[:, k, 0:HCOL],
                                  in_=x[k * P:(k + 1) * P, 0:HCOL])
            nc.sync.dma_start(
                out=x_sb[:, :, HCOL:COLS],
                in_=x[:, HCOL:COLS].rearrange("(k p) c -> p k c", p=P))
            nc.scalar.dma_start(
                out=w_sb[:, 1:, 0:WRAMP],
                in_=w[P:, 0:WRAMP].rearrange("(k p) c -> p k c", p=P))
            nc.scalar.dma_start(
                out=w_sb[:, :, WRAMP:WR],
                in_=w[:, WRAMP:WR].rearrange("(k p) c -> p k c", p=P))
            nc.scalar.dma_start(
                out=w_sb[:, :, WR:M_OUT],
                in_=w[:, WR:M_OUT].rearrange("(k p) c -> p k c", p=P))

            def copy_out(j, dst, src):
                if j % 2 == 0:
                    nc.vector.tensor_copy(out=dst, in_=src)
                else:
                    nc.scalar.copy(out=dst, in_=src)

            def group_mms(m, g, ps, k):
                for j in range(GN):
                    col = (g * GN + j) * NFREE
                    nc.tensor.matmul(ps[j], w_sb[:, k, m * P:(m + 1) * P],
                                     x_sb[:, k, col:col + NFREE],
                                     start=(k == 0), stop=(k == KC - 1))

            def group_finish(m, g, ps):
                # Output DMAs go out in half-group (256 KB) pieces split
                # across the two HWDGE rings: halves per-ring backlog so
                # the final drain is short, and the first half ships while
                # the second half is still being copied.
                o_sb = op.tile([P, HCOL], out_dt, tag="osb",
                               name=f"osb_{m}_{g}")
                qa, qb = ((nc.scalar, nc.sync) if (m + g) % 2 == 0
                          else (nc.sync, nc.scalar))
                for j in range(GN):
                    copy_out(j, o_sb[:, j * NFREE:(j + 1) * NFREE], ps[j])
                    if j == 1:
                        qa.dma_start(
                            out=out[m * P:(m + 1) * P,
                                    g * HCOL:g * HCOL + 2 * NFREE],
                            in_=o_sb[:, 0:2 * NFREE])
                qb.dma_start(
                    out=out[m * P:(m + 1) * P,
                            g * HCOL + 2 * NFREE:(g + 1) * HCOL],
                    in_=o_sb[:, 2 * NFREE:HCOL])

            def alloc_ps(m, g):
                return [pp.tile([P, NFREE], mybir.dt.float32, tag="ps",
                                name=f"ps_{m}_{g}_{j}") for j in range(GN)]

            # Ramp: m0/m1 group-0 blocks k-outer across all 8 PSUM banks,
            # tracking the x group-0 chunks as they land (8 matmuls per
            # chunk) so the PE never idles past the HAM re-throttle window.
            # k0 is further split j01-then-j23 to match the two x k0
            # half-chunk DMAs.
            ps_r = [alloc_ps(0, 0), alloc_ps(1, 0)]
            for half in range(2):
                for mi in range(2):
                    for j in (2 * half, 2 * half + 1):
                        nc.tensor.matmul(ps_r[mi][j],
                                         w_sb[:, 0, mi * P:(mi + 1) * P],
                                         x_sb[:, 0, j * NFREE:(j + 1) * NFREE],
                                         start=True, stop=False)
            for k in range(1, KC):
                for mi in range(2):
                    group_mms(mi, 0, ps_r[mi], k)
            for mi in range(2):
                group_finish(mi, 0, ps_r[mi])

            # Column-major sweeps: the rest of group 0 (m1..m3 dep-free on
            # the ramp-phase bytes, m4+ on the weight remainder that lands
            # behind them), then all of group 1.
            def sweep(m, g):
                ps = alloc_ps(m, g)
                for j in range(GN):
                    col = (g * GN + j) * NFREE
                    for k in range(KC):
                        nc.tensor.matmul(ps[j], w_sb[:, k, m * P:(m + 1) * P],
                                         x_sb[:, k, col:col + NFREE],
                                         start=(k == 0), stop=(k == KC - 1))
                if m == MT - 1 and g == 1:
                    # last group: stage/DMA per PSUM bank (4 x 128 KB
                    # pieces, alternating rings, both near-idle by now) so
                    # the kernel tail is one small copy + DMA, not 512 KB
                    # behind 4 serial copies.
                    for j2 in range(GN):
                        o_h = opt.tile([P, NFREE], out_dt,
                                       tag="osbt", name=f"osbt_{j2}")
                        copy_out(j2, o_h[:], ps[j2])
                        col0 = g * HCOL + j2 * NFREE
                        q = nc.scalar if j2 % 2 == 0 else nc.sync
                        q.dma_start(
                            out=out[m * P:(m + 1) * P, col0:col0 + NFREE],
                            in_=o_h[:])
                else:
                    group_finish(m, g, ps)

            for m in range(2, MT):
                sweep(m, 0)
            for m in range(MT):
                sweep(m, 1)
    nc.compile()
    return nc


def get_nc(dt_kind=DT_KIND, out_bf16=OUT_BF16):
    key = (dt_kind, out_bf16)
    if key not in _CACHE:
        _CACHE[key] = _build_nc(dt_kind, out_bf16)
    return _CACHE[key]


def build_weight(c_f):
    """(NSTACK, SIZE//2+1, 2) rfft coeffs -> circulant weight W (SIZE, M_OUT),
    W[k, s*SIZE + n] = c_s[(n - k) mod SIZE]."""
    c_f = np.asarray(c_f, np.float32)
    cf = c_f[..., 0].astype(np.float64) + 1j * c_f[..., 1].astype(np.float64)
    c = np.fft.irfft(cf, n=SIZE, axis=-1)            # (NSTACK, SIZE) float64
    idx = (np.arange(SIZE)[None, :] - np.arange(SIZE)[:, None]) % SIZE
    W = np.empty((SIZE, M_OUT), np.float32)
    for s in range(NSTACK):
        W[:, s * SIZE:(s + 1) * SIZE] = c[s][idx]
    return W


def _round_fp32r(a):
    """RNE-round fp32 to the fp32r storage format (e8m11 in the high 20
    bits of the word) — what the PE consumes in fp32r matmul mode."""
    u = np.ascontiguousarray(a, np.float32).view(np.uint32).copy()
    u += 0x7FF + ((u >> 12) & 1)
    u &= 0xFFFFF000
    return u.view(np.float32)


def make_in_maps(x, c_f, dt_kind=DT_KIND):
    x = np.asarray(x, np.float32)
    W = build_weight(c_f)
    if dt_kind == "bf16":
        import ml_dtypes
        cast = lambda a: np.ascontiguousarray(a).astype(ml_dtypes.bfloat16)
    elif dt_kind == "f32r":
        cast = _round_fp32r
    else:
        cast = lambda a: np.ascontiguousarray(a, np.float32)
    Wc = cast(W)
    in_maps = []
    for i in range(N_CORES):
        xs = (x[i * BPC:(i + 1) * BPC]
              .reshape(BPC, SIZE, HW)
              .transpose(1, 0, 2)
              .reshape(SIZE, COLS))
        in_maps.append({"x": cast(xs), "w": Wc})
    return in_maps


def assemble_output(per_core_outs):
    """list of 8 (M_OUT, COLS) -> (BATCH, M_OUT, 32, 32) fp32"""
    parts = [np.asarray(o, np.float32).reshape(M_OUT, BPC, HW).transpose(1, 0, 2)
             for o in per_core_outs]
    out = np.concatenate(parts, axis=0)               # (BATCH, M_OUT, HW)
    return np.ascontiguousarray(out.reshape(BATCH, M_OUT, 32, 32), np.float32)


def run(x, c_f, dt_kind=DT_KIND, **run_kwargs):
    """Returns (full_output, BassKernelResults)."""
    from concourse.bass_utils import run_bass_kernel_spmd
    nc = get_nc(dt_kind)
    in_maps = make_in_maps(x, c_f, dt_kind)
    res = run_bass_kernel_spmd(nc, in_maps, core_ids=list(range(N_CORES)),
                               **run_kwargs)
    out = assemble_output([r["out"] for r in res.results])
    return out, res


def kernel(input, c_f):
    out, _ = run(input, c_f)
    return out



# revision 22
# speedup vs baseline: 1.0387x; 1.0387x over previous
"""Circulant 1x1 conv (nn_Circulant1x1Conv) as a Trainium2 Bass kernel.

Math: the reference does, per spatial position r (N = batch*h*w rows):
    y[r, s*C + n] = irfft(rfft(x[r, :]) * cf[s])[n]  (circular convolution)
which is exactly a matmul  Y(N, 2048) = X(N, 512) @ W(512, 2048)  with
    W[k, s*C + n] = c_s[(n - k) mod C],   c_s = irfft(cf[s], n=C).

Crucially the native memory layouts are already transposed the right way:
  x[b] viewed as (C=512, h*w=1024) is X^T for that batch, and the output
  (nstack*C=2048, h*w) per batch is Y^T. So per batch:
      Out_b (2048, hw) = W^T @ X_b  ==  matmul(out, lhsT=W, rhs=X_b)
  on the tensor engine with zero data transposes anywhere.

Sharding: data-parallel over batch, 4 batches per core x 8 cores. Each core
computes a (2048, 4096) = (512, 2048)^T @ (512, 4096) matmul.

Precision knob DT_KIND:
  - "f32r": fp32 data, PE in fp32r (replicated/TF32-like) mode: 1 cycle/row
            at free-dim >= 256 per the cost model -> bf16-speed w/ fp32 inputs.
  - "bf16": inputs cast to bf16 on host; ~5e-3 rel error.
  - "f32":  exact fp32 matmul, 4 cycles/row (slow; debugging fallback).
"""

import numpy as np

SIZE = 512          # channels C (circulant size)
NSTACK = 4
BATCH = 32
HW = 32 * 32
N_CORES = 8
BPC = BATCH // N_CORES          # batches per core = 4
COLS = BPC * HW                 # moving free dim per core = 4096
M_OUT = NSTACK * SIZE           # output channels = 2048
P = 128
KC = SIZE // P                  # contraction chunks = 4
MT = M_OUT // P                 # output row tiles = 16
NFREE = 512                     # matmul moving free dim (1 PSUM bank fp32)
NT = COLS // NFREE              # moving chunks = 8
GN = 4                          # psum tiles per group (half of PSUM banks)
NG = NT // GN                   # groups per m-tile = 2

DT_KIND = "bf16"
OUT_BF16 = True     # DMA outputs as bf16 (half the write traffic); host upcasts

_CACHE = {}


def _build_nc(dt_kind, out_bf16=OUT_BF16):
    import concourse.bacc as bacc
    import concourse.tile as tile
    from concourse import mybir

    io_dt = {"bf16": mybir.dt.bfloat16,
             "f32r": mybir.dt.float32r,
             "f32": mybir.dt.float32}[dt_kind]
    out_dt = mybir.dt.bfloat16 if out_bf16 else mybir.dt.float32

    nc = bacc.Bacc("TRN2", name="circulant1x1")
    x = nc.dram_tensor("x", [SIZE, COLS], io_dt, kind="ExternalInput")
    w = nc.dram_tensor("w", [SIZE, M_OUT], io_dt, kind="ExternalInput")
    out = nc.dram_tensor("out", [M_OUT, COLS], out_dt,
                         kind="ExternalOutput")

    with tile.TileContext(nc) as tc:
        with (
            tc.tile_pool(name="xin", bufs=1) as xp,
            tc.tile_pool(name="win", bufs=1) as wp,
            tc.tile_pool(name="outp", bufs=8) as op,
            tc.tile_pool(name="outpt", bufs=4) as opt,
            tc.tile_pool(name="ps", bufs=8, space="PSUM") as pp,
        ):
            HCOL = COLS // NG                   # columns per group = 2048
            x_sb = xp.tile([P, KC, COLS], io_dt)
            w_sb = wp.tile([P, KC, M_OUT], io_dt)

            # All input DMAs (and all but the last two output groups) ride
            # the single Sync HWDGE ring: the DMA rings share the same 16
            # DMA engines, so splitting input streams across rings gains
            # no bandwidth - it only reorders arrivals. The ring also
            # ramps slowly (~130GB/s over its first ~1.5MB), so real
            # compute cannot start before ~13.5us no matter what; the
            # input order below simply matches the ramp's consumption
            # order. Input order: the m0..m3 weight columns (warmup fodder
            # + ramp weights), then all of x's group-0 half (the ramp
            # tracks these arrivals), then the remaining weight columns,
            # then x's group-1 half.
            WR = 4 * P                          # ramp weight columns
            # k0's ramp columns go first as a small separate piece so the
            # PE warmup (which reads them) can start ~2us earlier.
            nc.sync.dma_start(out=w_sb[:, 0, 0:WR], in_=w[0:P, 0:WR])
            nc.sync.dma_start(
                out=w_sb[:, 1:, 0:WR],
                in_=w[P:, 0:WR].rearrange("(k p) c -> p k c", p=P))
            for k in range(KC):
                nc.sync.dma_start(out=x_sb[:, k, 0:HCOL],
                                  in_=x[k * P:(k + 1) * P, 0:HCOL])
            for k in range(KC):
                nc.sync.dma_start(out=w_sb[:, k, WR:M_OUT],
                                  in_=w[k * P:(k + 1) * P, WR:M_OUT])
            for k in range(KC):
                nc.sync.dma_start(out=x_sb[:, k, HCOL:COLS],
                                  in_=x[k * P:(k + 1) * P, HCOL:COLS])

            # HAM warmup: dummy matmuls on the first weight piece while the
            # inputs stream in, so the PE hits K=8/8 (2.4 GHz) before the
            # real matmuls begin. Results discarded. Gating warmup on the
            # first small DMA keeps it phase-locked to the input stream -
            # an ungated early warmup ends too soon and lets the HAM
            # re-throttle before the first x chunk lands.
            for i in range(10):
                wps = pp.tile([P, NFREE], mybir.dt.float32, tag="ps",
                              name=f"warm_{i}")
                nc.tensor.matmul(wps, w_sb[:, 0, 0:P], w_sb[:, 0, 0:NFREE],
                                 start=True, stop=True)

            def copy_out(j, dst, src):
                if j % 2 == 0:
                    nc.vector.tensor_copy(out=dst, in_=src)
                else:
                    nc.scalar.copy(out=dst, in_=src)

            def group_mms(m, g, ps, k):
                for j in range(GN):
                    col = (g * GN + j) * NFREE
                    nc.tensor.matmul(ps[j], w_sb[:, k, m * P:(m + 1) * P],
                                     x_sb[:, k, col:col + NFREE],
                                     start=(k == 0), stop=(k == KC - 1))

            def group_finish(m, g, ps):
                # All mid-run outputs ride the Sync ring behind the inputs
                # (FIFO keeps input priority). The second-to-last group
                # goes to the otherwise-empty Scalar ring so the kernel
                # tail never waits behind the Sync ring's backlog.
                o_sb = op.tile([P, HCOL], out_dt, tag="osb",
                               name=f"osb_{m}_{g}")
                for j in range(GN):
                    copy_out(j, o_sb[:, j * NFREE:(j + 1) * NFREE], ps[j])
                q = nc.scalar if (m == MT - 1 and g == 1) or \
                    (m == MT - 2 and g == 1) else nc.sync
                q.dma_start(
                    out=out[m * P:(m + 1) * P, g * HCOL:(g + 1) * HCOL],
                    in_=o_sb[:])

            def alloc_ps(m, g):
                return [pp.tile([P, NFREE], mybir.dt.float32, tag="ps",
                                name=f"ps_{m}_{g}_{j}") for j in range(GN)]

            # Ramp: m0/m1 group-0 blocks k-outer across all 8 PSUM banks,
            # tracking the x group-0 chunks as they land (8 matmuls per
            # chunk) so the PE never idles past the HAM re-throttle window.
            ps_r = [alloc_ps(0, 0), alloc_ps(1, 0)]
            for k in range(KC):
                for mi in range(2):
                    group_mms(mi, 0, ps_r[mi], k)
            for mi in range(2):
                group_finish(mi, 0, ps_r[mi])

            # Column-major sweeps: the rest of group 0 (m1..m3 dep-free on
            # the ramp-phase bytes, m4+ on the weight remainder that lands
            # behind them), then all of group 1.
            def sweep(m, g):
                ps = alloc_ps(m, g)
                for j in range(GN):
                    col = (g * GN + j) * NFREE
                    for k in range(KC):
                        nc.tensor.matmul(ps[j], w_sb[:, k, m * P:(m + 1) * P],
                                         x_sb[:, k, col:col + NFREE],
                                         start=(k == 0), stop=(k == KC - 1))
                if m == MT - 1 and g == 1:
                    # last group: stage/DMA per PSUM bank (4 x 128 KB
                    # pieces) on the near-empty Scalar ring, so the kernel
                    # tail is one small copy + DMA, not 512 KB behind 4
                    # serial copies and the Sync ring's output backlog.
                    for j2 in range(GN):
                        o_h = opt.tile([P, NFREE], out_dt,
                                       tag="osbt", name=f"osbt_{j2}")
                        copy_out(j2, o_h[:], ps[j2])
                        col0 = g * HCOL + j2 * NFREE
                        nc.scalar.dma_start(
                            out=out[m * P:(m + 1) * P, col0:col0 + NFREE],
                            in_=o_h[:])
                else:
                    group_finish(m, g, ps)

            for m in range(2, MT):
                sweep(m, 0)
            for m in range(MT):
                sweep(m, 1)
    nc.compile()
    return nc


def get_nc(dt_kind=DT_KIND, out_bf16=OUT_BF16):
    key = (dt_kind, out_bf16)
    if key not in _CACHE:
        _CACHE[key] = _build_nc(dt_kind, out_bf16)
    return _CACHE[key]


def build_weight(c_f):
    """(NSTACK, SIZE//2+1, 2) rfft coeffs -> circulant weight W (SIZE, M_OUT),
    W[k, s*SIZE + n] = c_s[(n - k) mod SIZE]."""
    c_f = np.asarray(c_f, np.float32)
    cf = c_f[..., 0].astype(np.float64) + 1j * c_f[..., 1].astype(np.float64)
    c = np.fft.irfft(cf, n=SIZE, axis=-1)            # (NSTACK, SIZE) float64
    idx = (np.arange(SIZE)[None, :] - np.arange(SIZE)[:, None]) % SIZE
    W = np.empty((SIZE, M_OUT), np.float32)
    for s in range(NSTACK):
        W[:, s * SIZE:(s + 1) * SIZE] = c[s][idx]
    return W


def _round_fp32r(a):
    """RNE-round fp32 to the fp32r storage format (e8m11 in the high 20
    bits of the word) — what the PE consumes in fp32r matmul mode."""
    u = np.ascontiguousarray(a, np.float32).view(np.uint32).copy()
    u += 0x7FF + ((u >> 12) & 1)
    u &= 0xFFFFF000
    return u.view(np.float32)


def make_in_maps(x, c_f, dt_kind=DT_KIND):
    x = np.asarray(x, np.float32)
    W = build_weight(c_f)
    if dt_kind == "bf16":
        import ml_dtypes
        cast = lambda a: np.ascontiguousarray(a).astype(ml_dtypes.bfloat16)
    elif dt_kind == "f32r":
        cast = _round_fp32r
    else:
        cast = lambda a: np.ascontiguousarray(a, np.float32)
    Wc = cast(W)
    in_maps = []
    for i in range(N_CORES):
        xs = (x[i * BPC:(i + 1) * BPC]
              .reshape(BPC, SIZE, HW)
              .transpose(1, 0, 2)
              .reshape(SIZE, COLS))
        in_maps.append({"x": cast(xs), "w": Wc})
    return in_maps


def assemble_output(per_core_outs):
    """list of 8 (M_OUT, COLS) -> (BATCH, M_OUT, 32, 32) fp32"""
    parts = [np.asarray(o, np.float32).reshape(M_OUT, BPC, HW).transpose(1, 0, 2)
             for o in per_core_outs]
    out = np.concatenate(parts, axis=0)               # (BATCH, M_OUT, HW)
    return np.ascontiguousarray(out.reshape(BATCH, M_OUT, 32, 32), np.float32)


def run(x, c_f, dt_kind=DT_KIND, **run_kwargs):
    """Returns (full_output, BassKernelResults)."""
    from concourse.bass_utils import run_bass_kernel_spmd
    nc = get_nc(dt_kind)
    in_maps = make_in_maps(x, c_f, dt_kind)
    res = run_bass_kernel_spmd(nc, in_maps, core_ids=list(range(N_CORES)),
                               **run_kwargs)
    out = assemble_output([r["out"] for r in res.results])
    return out, res


def kernel(input, c_f):
    out, _ = run(input, c_f)
    return out

